# revision 26
# baseline (speedup 1.0000x reference)
"""Malvar-He-Cutler demosaic on 8 Trainium2 NeuronCores.

Strategy (W-sharding, bf16, host passthrough/clip):
  - Host reflect-pads x, converts to bf16, column-shards into 8 slices of
    768 cols (+2 halo each side), and splits each shard into column-parity
    planes: xp [4100, 772] = [even-col plane 386 | odd-col plane 386].
    Parity-split planes make every matmul moving operand and DVE operand
    a packed (stride-1) bf16 slice - full PE stream rate (the stride-2
    f32 moving operands of the original design ran the PE ~3x slower).
  - Per core, row tiles of 124 output rows; X [128, 772] loaded in natural
    row order (partition p = padded row r0+p) by one SWDGE DMA.
  - The MHC kernels have symmetric columns [a,b,c,b,a]; DVE pre-adds the
    outer column pairs (A = s[c]+s[c+2], B = t[c]+t[c+1] per parity) with
    two fused 2-segment ops issued alongside the loads, so each of the 4
    conv maps needs only 3 matmul taps (center/A/B) instead of 5: 12
    matmul passes per tile, PSUM-accumulated. Banded bf16 stationaries do
    the vertical 5-tap conv and pack even output rows at partitions 0-61,
    odd at 62-123.
  - The passthrough channel (1 of every pixel's 3 channels is x itself)
    and the final clip are done on the HOST: the device only computes and
    ships the 2 interpolated channels per pixel as bf16 (-2/3 store
    bytes vs f32 RGB).
  - Maps are computed in 2-bank psum pairs; one ACT activation-copy per
    pair evicts PSUM f32 -> asm bf16 (packed writes, no clip). A single
    SWDGE store per tile writes asm to out [4096, 4*384] = per-row
    sections [E1|E2|O1|O2] via a (parity, row) rearranged DRAM view;
    host clips, interleaves with x, and assembles [4096, 6144, 3] f32.
"""

import numpy as np
import ml_dtypes

H, W = 4096, 6144
NCORES = 8
CS = W // NCORES          # 768 cols per core
NC2 = CS // 2             # 384 cols per parity
PHW = NC2 + 2             # 386 parity-plane width (with halo)
XW = 2 * PHW              # 772
TILE_R = 124              # output rows per tile
OUTW = 4 * NC2            # 1536 device-output elems per row (bf16)

_PROGRAMS = {}


def _build_program(sym=True):
    from concourse import bacc, mybir, tile

    f32 = mybir.dt.float32
    bf16 = mybir.dt.bfloat16
    ntaps = 3 if sym else 5

    nc = bacc.Bacc(None, target_bir_lowering=False, debug=True)
    xp_d = nc.dram_tensor("xp", [H + 4, XW], bf16, kind="ExternalInput")
    w_d = nc.dram_tensor("wst", [128, 4 * ntaps * 128], bf16, kind="ExternalInput")
    out_d = nc.dram_tensor("out", [H, OUTW], bf16, kind="ExternalOutput")

    r0s = [TILE_R * i for i in range(H // TILE_R)]
    if r0s[-1] + TILE_R < H:
        r0s.append(H - TILE_R)

    copy_f = mybir.ActivationFunctionType.Copy
    add = mybir.AluOpType.add
    mult = mybir.AluOpType.mult
    mn, mx = mybir.AluOpType.min, mybir.AluOpType.max

    STORE_SKEW = 2  # store tile i while computing tile i+2
    LOAD_AHEAD = 4
    PSB = 512  # psum bank stride in f32 elements

    with tile.TileContext(nc) as tc:
        with tc.tile_pool(name="wpool", bufs=1) as wpool, \
             tc.tile_pool(name="xpool", bufs=LOAD_AHEAD + 5) as xpool, \
             tc.tile_pool(name="spool", bufs=LOAD_AHEAD + 5) as spool, \
             tc.tile_pool(name="apool", bufs=STORE_SKEW + 6) as apool, \
             tc.tile_pool(name="ppool", bufs=2, space="PSUM") as ppool:

            wt = wpool.tile([128, 4 * ntaps * 128], bf16, name="wt")
            nc.sync.dma_start(out=wt[:], in_=w_d.ap())

            def issue_load(r0):
                Xt = xpool.tile([128, XW], bf16, name="X", tag="X")
                nc.gpsimd.dma_start(out=Xt[:], in_=xp_d[r0 : r0 + 128, :])
                if not sym:
                    return Xt, None
                # pre-adds issued with the load: DVE runs only these, in tile
                # order, so it pipelines ahead of the PE unimpeded.
                # Fused via 2-segment APs; S layout = [A_e | B_e | B_o | A_o]:
                #   A_e = xpe[c]+xpe[c+2], B_e = xpo[c]+xpo[c+1]
                #   B_o = xpe[c+1]+xpe[c+2], A_o = xpo[c]+xpo[c+2]
                S = spool.tile([128, 4 * NC2], bf16, name="S", tag="S")
                stt = nc.vector.scalar_tensor_tensor
                seg = lambda lo, hi, a, b: (
                    Xt[:, lo:hi].rearrange("p (s f) -> p s f", s=2)[:, :, a:b]
                )
                stt(S[:, 0 : 2 * NC2], seg(0, 772, 0, NC2), 1.0,
                    seg(2, 772, 0, NC2), op0=mult, op1=add)
                stt(S[:, 2 * NC2 : 4 * NC2], seg(1, 771, 0, NC2), 1.0,
                    seg(0, 772, 2, NC2 + 2), op0=mult, op1=add)
                return Xt, S

            def psview(ps, p0, p1):
                # [p, 4 maps, 384] f32 view of the 4-bank psum tile
                return ps[p0:p1, :].rearrange("p (m f) -> p m f", m=4)[:, :, 0:NC2]

            def store(r0, asm):
                if r0 % TILE_R == 0:
                    # single DMA: DRAM rows reordered (even block, odd block)
                    # to match asm partitions 0-61 / 62-123
                    oview = out_d[r0 : r0 + TILE_R, :].rearrange(
                        "(i h) f -> h i f", h=2
                    )
                    nc.gpsimd.dma_start(out=oview, in_=asm[0:124, :])
                else:
                    # overlap tile: emit only the rows no earlier tile wrote
                    new0 = (r0s[-2] + TILE_R - r0) // 2
                    nc.gpsimd.dma_start(
                        out=out_d[r0 + 2 * new0 : r0 + TILE_R : 2, :],
                        in_=asm[new0:62, :],
                    )
                    nc.gpsimd.dma_start(
                        out=out_d[r0 + 2 * new0 + 1 : r0 + TILE_R : 2, :],
                        in_=asm[62 + new0 : 124, :],
                    )

            pending = []
            loaded = {k: issue_load(r0s[k]) for k in range(min(LOAD_AHEAD + 1, len(r0s)))}
            for j, r0 in enumerate(r0s):
                X, S = loaded.pop(j)
                if j + LOAD_AHEAD + 1 < len(r0s):
                    loaded[j + LOAD_AHEAD + 1] = issue_load(r0s[j + LOAD_AHEAD + 1])

                if sym:
                    mov_e = [X[:, 1 : NC2 + 1], S[:, 0:NC2], S[:, NC2 : 2 * NC2]]
                    mov_o = [X[:, PHW + 1 : PHW + NC2 + 1], S[:, 3 * NC2 : 4 * NC2],
                             S[:, 2 * NC2 : 3 * NC2]]
                else:
                    mov_e = [X[:, 0:NC2], X[:, 1 : NC2 + 1], X[:, 2 : NC2 + 2],
                             X[:, PHW : PHW + NC2], X[:, PHW + 1 : PHW + NC2 + 1]]
                    mov_o = [X[:, PHW : PHW + NC2], X[:, PHW + 1 : PHW + NC2 + 1],
                             X[:, PHW + 2 : PHW + NC2 + 2], X[:, 1 : NC2 + 1],
                             X[:, 2 : NC2 + 2]]
                movs = [mov_e, mov_e, mov_o, mov_o]  # E1, E2, O1, O2

                asm = apool.tile([128, OUTW], bf16, name="asm", tag="asm")
                pstiles = [
                    ppool.tile([128, 2 * PSB], f32, name=f"ps{p}", tag=f"ps{p}")
                    for p in range(2)
                ]

                def mm(m, s_i):
                    ps = pstiles[m // 2]
                    nc.tensor.matmul(
                        ps[:, (m % 2) * PSB : (m % 2) * PSB + NC2],
                        lhsT=wt[:, (m * ntaps + s_i) * 128 : (m * ntaps + s_i + 1) * 128],
                        rhs=movs[m][s_i],
                        start=(s_i == 0),
                        stop=(s_i == ntaps - 1),
                    )

                def evict(pair):
                    src = pstiles[pair][0:124, :].rearrange(
                        "p (b f) -> p b f", b=2
                    )[:, :, 0:NC2]
                    dst = asm[0:124, 2 * pair * NC2 : 2 * (pair + 1) * NC2]
                    nc.scalar.activation(dst, src, copy_f)

                for m in range(4):
                    for s_i in range(ntaps):
                        mm(m, s_i)
                    if m % 2 == 1:
                        evict(m // 2)

                pending.append((r0, asm))
                if len(pending) > STORE_SKEW:
                    store(*pending.pop(0))
            for item in pending:
                store(*item)
    nc.compile()
    return nc


def _get_program(sym):
    if sym not in _PROGRAMS:
        _PROGRAMS[sym] = _build_program(sym)
    return _PROGRAMS[sym]


def _build_stationary(kern, sym):
    """kern: [4,5,5] f32 -> W [128, 4*ntaps*128] bf16 (lhsT per tap)."""
    groups = [(0, 2), (3, 1), (1, 3), (2, 0)]  # (even-row kernel, odd-row kernel)
    ntaps = 3 if sym else 5
    Wm = np.zeros((4 * ntaps, 128, 128), np.float32)
    t = np.arange(62)
    for m, (ka, kb) in enumerate(groups):
        if sym:
            profs_a = [kern[ka][:, 2], kern[ka][:, 0], kern[ka][:, 1]]
            profs_b = [kern[kb][:, 2], kern[kb][:, 0], kern[kb][:, 1]]
        else:
            # stream order must match mov_e/mov_o: [p0, p1(center), p2, q0, q1]
            # e-maps: xpe[c]=col0, xpe[c+1]=col2, xpe[c+2]=col4, xpo[c]=col1,
            # xpo[c+1]=col3 (same relative cols for o-maps by symmetry of
            # window construction)
            cols = [0, 2, 4, 1, 3]
            profs_a = [kern[ka][:, c] for c in cols]
            profs_b = [kern[kb][:, c] for c in cols]
        for s in range(ntaps):
            Wq = Wm[m * ntaps + s]
            for di in range(5):
                Wq[2 * t + di, t] += profs_a[s][di]        # even out rows -> p 0-61
                Wq[2 * t + 1 + di, 62 + t] += profs_b[s][di]  # odd out rows -> p 62-123
    out = np.ascontiguousarray(Wm.transpose(1, 0, 2).reshape(128, 4 * ntaps * 128))
    return out.astype(ml_dtypes.bfloat16)


def kernel(x, kernels, _trace=False):
    from concourse.bass_utils import run_bass_kernel_spmd

    x = np.asarray(x, dtype=np.float32)
    kern = np.asarray(kernels, dtype=np.float32).reshape(4, 5, 5)
    sym = bool(
        np.array_equal(kern[:, :, 0], kern[:, :, 4])
        and np.array_equal(kern[:, :, 1], kern[:, :, 3])
    )
    wst = _build_stationary(kern, sym)
    xpad = np.pad(x, 2, mode="reflect").astype(ml_dtypes.bfloat16)

    in_maps = []
    for c in range(NCORES):
        sh = xpad[:, c * CS : c * CS + CS + 4]  # [4100, 772]
        shp = np.concatenate([sh[:, 0::2], sh[:, 1::2]], axis=1)
        in_maps.append({"xp": np.ascontiguousarray(shp), "wst": wst})

    nc = _get_program(sym)
    res = run_bass_kernel_spmd(nc, in_maps, list(range(NCORES)), trace=_trace)

    # secs[s] = [H, 8*384] grid over (core, parity-col index) = full half-res col grid
    secs = np.stack(
        [np.asarray(res.results[c]["out"]).reshape(H, 4, NC2) for c in range(NCORES)],
        axis=2,
    ).astype(np.float32)  # [H, 4, 8, 384]
    E1 = np.clip(secs[:, 0].reshape(H, W // 2), 0.0, 1.0)
    E2 = np.clip(secs[:, 1].reshape(H, W // 2), 0.0, 1.0)
    O1 = np.clip(secs[:, 2].reshape(H, W // 2), 0.0, 1.0)
    O2 = np.clip(secs[:, 3].reshape(H, W // 2), 0.0, 1.0)
    xc = np.clip(x, 0.0, 1.0)

    out = np.empty((H, W, 3), np.float32)
    # R channel
    out[0::2, 0::2, 0] = xc[0::2, 0::2]
    out[1::2, 0::2, 0] = E1[1::2]
    out[:, 1::2, 0] = O1
    # G channel
    out[0::2, 0::2, 1] = E1[0::2]
    out[1::2, 0::2, 1] = xc[1::2, 0::2]
    out[0::2, 1::2, 1] = xc[0::2, 1::2]
    out[1::2, 1::2, 1] = O2[1::2]
    # B channel
    out[:, 0::2, 2] = E2
    out[0::2, 1::2, 2] = O2[0::2]
    out[1::2, 1::2, 2] = xc[1::2, 1::2]

    if _trace:
        return out, res
    return out


# revision 28
# speedup vs baseline: 1.0269x; 1.0269x over previous
"""Malvar-He-Cutler demosaic on 8 Trainium2 NeuronCores.

Strategy (W-sharding, bf16, host passthrough/clip):
  - Host reflect-pads x, converts to bf16, column-shards into 8 slices of
    768 cols (+2 halo each side), and splits each shard into column-parity
    planes: xp [4100, 772] = [even-col plane 386 | odd-col plane 386].
    Parity-split planes make every matmul moving operand and DVE operand
    a packed (stride-1) bf16 slice - full PE stream rate (the stride-2
    f32 moving operands of the original design ran the PE ~3x slower).
  - Per core, row tiles of 124 output rows; X [128, 772] loaded in natural
    row order (partition p = padded row r0+p) by one SWDGE DMA.
  - The MHC kernels have symmetric columns [a,b,c,b,a]; DVE pre-adds the
    outer column pairs (A = s[c]+s[c+2], B = t[c]+t[c+1] per parity) with
    two fused 2-segment ops issued alongside the loads, so each of the 4
    conv maps needs only 3 matmul taps (center/A/B) instead of 5: 12
    matmul passes per tile, PSUM-accumulated. Banded bf16 stationaries do
    the vertical 5-tap conv and pack even output rows at partitions 0-61,
    odd at 62-123.
  - The passthrough channel (1 of every pixel's 3 channels is x itself)
    and the final clip are done on the HOST: the device only computes and
    ships the 2 interpolated channels per pixel as bf16 (-2/3 store
    bytes vs f32 RGB).
  - Maps are computed in 2-bank psum pairs; one ACT activation-copy per
    pair evicts PSUM f32 -> asm bf16 (packed writes, no clip). A single
    SWDGE store per tile writes asm to out [4096, 4*384] = per-row
    sections [E1|E2|O1|O2] via a (parity, row) rearranged DRAM view;
    host clips, interleaves with x, and assembles [4096, 6144, 3] f32.
"""

import numpy as np
import ml_dtypes

H, W = 4096, 6144
NCORES = 8
CS = W // NCORES          # 768 cols per core
NC2 = CS // 2             # 384 cols per parity
PHW = NC2 + 2             # 386 parity-plane width (with halo)
XW = 2 * PHW              # 772
TILE_R = 124              # output rows per tile
OUTW = 4 * NC2            # 1536 device-output elems per row (bf16)

_PROGRAMS = {}


def _build_program(sym=True):
    from concourse import bacc, mybir, tile

    f32 = mybir.dt.float32
    bf16 = mybir.dt.bfloat16
    ntaps = 3 if sym else 5

    nc = bacc.Bacc(None, target_bir_lowering=False, debug=True)
    xp_d = nc.dram_tensor("xp", [H + 4, XW], bf16, kind="ExternalInput")
    w_d = nc.dram_tensor("wst", [128, 4 * ntaps * 128], bf16, kind="ExternalInput")
    out_d = nc.dram_tensor("out", [H, OUTW], bf16, kind="ExternalOutput")

    r0s = [TILE_R * i for i in range(H // TILE_R)]
    if r0s[-1] + TILE_R < H:
        r0s.append(H - TILE_R)

    copy_f = mybir.ActivationFunctionType.Copy
    add = mybir.AluOpType.add
    mult = mybir.AluOpType.mult
    mn, mx = mybir.AluOpType.min, mybir.AluOpType.max

    STORE_SKEW = 1  # store tile i while computing tile i+1
    LOAD_AHEAD = 4
    PSB = 512  # psum bank stride in f32 elements

    with tile.TileContext(nc) as tc:
        with tc.tile_pool(name="wpool", bufs=1) as wpool, \
             tc.tile_pool(name="xpool", bufs=LOAD_AHEAD + 5) as xpool, \
             tc.tile_pool(name="spool", bufs=LOAD_AHEAD + 5) as spool, \
             tc.tile_pool(name="apool", bufs=STORE_SKEW + 11) as apool, \
             tc.tile_pool(name="ppool", bufs=2, space="PSUM") as ppool:

            wt = wpool.tile([128, 4 * ntaps * 128], bf16, name="wt")
            nc.sync.dma_start(out=wt[:], in_=w_d.ap())

            def issue_load(r0):
                Xt = xpool.tile([128, XW], bf16, name="X", tag="X")
                nc.gpsimd.dma_start(out=Xt[:], in_=xp_d[r0 : r0 + 128, :])
                if not sym:
                    return Xt, None
                # pre-adds issued with the load: DVE runs only these, in tile
                # order, so it pipelines ahead of the PE unimpeded.
                # Fused via 2-segment APs; S layout = [A_e | B_e | B_o | A_o]:
                #   A_e = xpe[c]+xpe[c+2], B_e = xpo[c]+xpo[c+1]
                #   B_o = xpe[c+1]+xpe[c+2], A_o = xpo[c]+xpo[c+2]
                S = spool.tile([128, 4 * NC2], bf16, name="S", tag="S")
                stt = nc.vector.scalar_tensor_tensor
                seg = lambda lo, hi, a, b: (
                    Xt[:, lo:hi].rearrange("p (s f) -> p s f", s=2)[:, :, a:b]
                )
                stt(S[:, 0 : 2 * NC2], seg(0, 772, 0, NC2), 1.0,
                    seg(2, 772, 0, NC2), op0=mult, op1=add)
                stt(S[:, 2 * NC2 : 4 * NC2], seg(1, 771, 0, NC2), 1.0,
                    seg(0, 772, 2, NC2 + 2), op0=mult, op1=add)
                return Xt, S

            def psview(ps, p0, p1):
                # [p, 4 maps, 384] f32 view of the 4-bank psum tile
                return ps[p0:p1, :].rearrange("p (m f) -> p m f", m=4)[:, :, 0:NC2]

            def store(r0, asm):
                if r0 % TILE_R == 0:
                    # single DMA: DRAM rows reordered (even block, odd block)
                    # to match asm partitions 0-61 / 62-123
                    oview = out_d[r0 : r0 + TILE_R, :].rearrange(
                        "(i h) f -> h i f", h=2
                    )
                    nc.gpsimd.dma_start(out=oview, in_=asm[0:124, :])
                else:
                    # overlap tile: emit only the rows no earlier tile wrote
                    new0 = (r0s[-2] + TILE_R - r0) // 2
                    nc.gpsimd.dma_start(
                        out=out_d[r0 + 2 * new0 : r0 + TILE_R : 2, :],
                        in_=asm[new0:62, :],
                    )
                    nc.gpsimd.dma_start(
                        out=out_d[r0 + 2 * new0 + 1 : r0 + TILE_R : 2, :],
                        in_=asm[62 + new0 : 124, :],
                    )

            pending = []
            loaded = {k: issue_load(r0s[k]) for k in range(min(LOAD_AHEAD + 1, len(r0s)))}
            for j, r0 in enumerate(r0s):
                X, S = loaded.pop(j)
                if j + LOAD_AHEAD + 1 < len(r0s):
                    loaded[j + LOAD_AHEAD + 1] = issue_load(r0s[j + LOAD_AHEAD + 1])

                if sym:
                    mov_e = [X[:, 1 : NC2 + 1], S[:, 0:NC2], S[:, NC2 : 2 * NC2]]
                    mov_o = [X[:, PHW + 1 : PHW + NC2 + 1], S[:, 3 * NC2 : 4 * NC2],
                             S[:, 2 * NC2 : 3 * NC2]]
                else:
                    mov_e = [X[:, 0:NC2], X[:, 1 : NC2 + 1], X[:, 2 : NC2 + 2],
                             X[:, PHW : PHW + NC2], X[:, PHW + 1 : PHW + NC2 + 1]]
                    mov_o = [X[:, PHW : PHW + NC2], X[:, PHW + 1 : PHW + NC2 + 1],
                             X[:, PHW + 2 : PHW + NC2 + 2], X[:, 1 : NC2 + 1],
                             X[:, 2 : NC2 + 2]]
                movs = [mov_e, mov_e, mov_o, mov_o]  # E1, E2, O1, O2

                asm = apool.tile([128, OUTW], bf16, name="asm", tag="asm")
                pstiles = [
                    ppool.tile([128, 2 * PSB], f32, name=f"ps{p}", tag=f"ps{p}")
                    for p in range(2)
                ]

                def mm(m, s_i):
                    ps = pstiles[m // 2]
                    nc.tensor.matmul(
                        ps[:, (m % 2) * PSB : (m % 2) * PSB + NC2],
                        lhsT=wt[:, (m * ntaps + s_i) * 128 : (m * ntaps + s_i + 1) * 128],
                        rhs=movs[m][s_i],
                        start=(s_i == 0),
                        stop=(s_i == ntaps - 1),
                    )

                def evict(pair):
                    src = pstiles[pair][0:124, :].rearrange(
                        "p (b f) -> p b f", b=2
                    )[:, :, 0:NC2]
                    dst = asm[0:124, 2 * pair * NC2 : 2 * (pair + 1) * NC2]
                    nc.scalar.activation(dst, src, copy_f)

                for m in range(4):
                    for s_i in range(ntaps):
                        mm(m, s_i)
                    if m % 2 == 1:
                        evict(m // 2)

                pending.append((r0, asm))
                if len(pending) > STORE_SKEW:
                    store(*pending.pop(0))
            for item in pending:
                store(*item)
    nc.compile()
    return nc


def _get_program(sym):
    if sym not in _PROGRAMS:
        _PROGRAMS[sym] = _build_program(sym)
    return _PROGRAMS[sym]


def _build_stationary(kern, sym):
    """kern: [4,5,5] f32 -> W [128, 4*ntaps*128] bf16 (lhsT per tap)."""
    groups = [(0, 2), (3, 1), (1, 3), (2, 0)]  # (even-row kernel, odd-row kernel)
    ntaps = 3 if sym else 5
    Wm = np.zeros((4 * ntaps, 128, 128), np.float32)
    t = np.arange(62)
    for m, (ka, kb) in enumerate(groups):
        if sym:
            profs_a = [kern[ka][:, 2], kern[ka][:, 0], kern[ka][:, 1]]
            profs_b = [kern[kb][:, 2], kern[kb][:, 0], kern[kb][:, 1]]
        else:
            # stream order must match mov_e/mov_o: [p0, p1(center), p2, q0, q1]
            # e-maps: xpe[c]=col0, xpe[c+1]=col2, xpe[c+2]=col4, xpo[c]=col1,
            # xpo[c+1]=col3 (same relative cols for o-maps by symmetry of
            # window construction)
            cols = [0, 2, 4, 1, 3]
            profs_a = [kern[ka][:, c] for c in cols]
            profs_b = [kern[kb][:, c] for c in cols]
        for s in range(ntaps):
            Wq = Wm[m * ntaps + s]
            for di in range(5):
                Wq[2 * t + di, t] += profs_a[s][di]        # even out rows -> p 0-61
                Wq[2 * t + 1 + di, 62 + t] += profs_b[s][di]  # odd out rows -> p 62-123
    out = np.ascontiguousarray(Wm.transpose(1, 0, 2).reshape(128, 4 * ntaps * 128))
    return out.astype(ml_dtypes.bfloat16)


def kernel(x, kernels, _trace=False):
    from concourse.bass_utils import run_bass_kernel_spmd

    x = np.asarray(x, dtype=np.float32)
    kern = np.asarray(kernels, dtype=np.float32).reshape(4, 5, 5)
    sym = bool(
        np.array_equal(kern[:, :, 0], kern[:, :, 4])
        and np.array_equal(kern[:, :, 1], kern[:, :, 3])
    )
    wst = _build_stationary(kern, sym)
    xpad = np.pad(x, 2, mode="reflect").astype(ml_dtypes.bfloat16)

    in_maps = []
    for c in range(NCORES):
        sh = xpad[:, c * CS : c * CS + CS + 4]  # [4100, 772]
        shp = np.concatenate([sh[:, 0::2], sh[:, 1::2]], axis=1)
        in_maps.append({"xp": np.ascontiguousarray(shp), "wst": wst})

    nc = _get_program(sym)
    res = run_bass_kernel_spmd(nc, in_maps, list(range(NCORES)), trace=_trace)

    # secs[s] = [H, 8*384] grid over (core, parity-col index) = full half-res col grid
    secs = np.stack(
        [np.asarray(res.results[c]["out"]).reshape(H, 4, NC2) for c in range(NCORES)],
        axis=2,
    ).astype(np.float32)  # [H, 4, 8, 384]
    E1 = np.clip(secs[:, 0].reshape(H, W // 2), 0.0, 1.0)
    E2 = np.clip(secs[:, 1].reshape(H, W // 2), 0.0, 1.0)
    O1 = np.clip(secs[:, 2].reshape(H, W // 2), 0.0, 1.0)
    O2 = np.clip(secs[:, 3].reshape(H, W // 2), 0.0, 1.0)
    xc = np.clip(x, 0.0, 1.0)

    out = np.empty((H, W, 3), np.float32)
    # R channel
    out[0::2, 0::2, 0] = xc[0::2, 0::2]
    out[1::2, 0::2, 0] = E1[1::2]
    out[:, 1::2, 0] = O1
    # G channel
    out[0::2, 0::2, 1] = E1[0::2]
    out[1::2, 0::2, 1] = xc[1::2, 0::2]
    out[0::2, 1::2, 1] = xc[0::2, 1::2]
    out[1::2, 1::2, 1] = O2[1::2]
    # B channel
    out[:, 0::2, 2] = E2
    out[0::2, 1::2, 2] = O2[0::2]
    out[1::2, 1::2, 2] = xc[1::2, 1::2]

    if _trace:
        return out, res
    return out
